# revision 4
# baseline (speedup 1.0000x reference)
"""Trainium2 Bass kernel for nn_ChebyshevLayer (gnn_message_passing) — v3 (PE SpMM).

Sharding/structure (8 NeuronCores, SPMD, 3 launches):
- X0 = transpose(x,(1,2,0)).reshape(M, Fin*N) -> [50000, 128] bf16, padded to
  50176 rows; kept in HBM in natural row-major layout (256B rows).
- Rows grouped in 32-row WINDOWS (1568 global). Window w needs its nnz's
  X[col] rows; cols split in halves (int16 gather range). Per (window, half)
  the slots are padded to a multiple of 128 (tiles). Windows are dealt to the
  8 cores BY CLASS (tA, tB) so every core runs an identical schedule (one
  NEFF); per-class remainders are padded with dummy windows.
- SpMM: per group of 16 windows (one PSUM bank [128,512] f32): two
  dma_gathers (half A / half B, natural layout: slot->partition, 4 SWDGE
  queues round-robin), then per 128-slot tile one PE matmul
  psum[feat, 32] += G_tile^T[feat, slots] @ B_tile[slots, 32], where
  B[s, r] = alpha*val[s] if slot s targets local row r (bf16, streamed).
  DVE then computes xn[:, grp] = psum - xprev[:, grp] (Chebyshev recurrence;
  alpha=2 folded into vals for iterations 2,3; xprev=0 on iteration 1).
- Launch 3 additionally fuses the einsum: psum[m,256] = sum_k T_k^T @ What_k
  (block-diag What: [(f,n), (n,o)]) + ones^T@bias, out bf16 [R,256].
- Host assembles the full X between launches (pure data movement).
"""

import numpy as np
import ml_dtypes

import concourse.bacc as bacc
import concourse.mybir as mybir
from concourse.tile import TileContext
from concourse.bass_utils import run_bass_kernel_spmd
from concourse.library_config import mlp

N, M, FIN, FOUT, KCH = 8, 50000, 16, 32, 4
WIN = 44                   # rows per window
NWING = -(-M // WIN)       # global windows
MP = NWING * WIN           # padded rows
HALF = MP // 2             # col half for int16 gather indices
NCORE = 8
NQUEUES = 4
GRP = 11                   # windows per psum bank group (GRP*WIN*4B <= 2KB)
BF16 = ml_dtypes.bfloat16


def _ceil(a, b):
    return -(-a // b)


class Plan:
    pass


def build_plan(rows, cols):
    """Window classes, per-core schedule (shared shape), slot/B layout."""
    p = Plan()
    win = rows // WIN
    isA = cols < HALF
    cntA = np.bincount(win[isA], minlength=NWING).astype(np.int64)
    cntB = np.bincount(win[~isA], minlength=NWING).astype(np.int64)
    tA = np.maximum(1, _ceil(cntA, 128))     # >=1 so every window is written
    tB = _ceil(cntB, 128)
    cls = tA * 16 + tB
    order = np.argsort(cls, kind="stable")
    ucls, starts, ncls = np.unique(cls[order], return_index=True,
                                   return_counts=True)
    per_core = _ceil(ncls, NCORE)            # windows of this class per core
    NWp = int(per_core.sum())                # scheduled windows per core
    # schedule (same for all cores): sched position s -> (tA, tB)
    sched_tA = np.repeat((ucls // 16).astype(np.int64), per_core)
    sched_tB = np.repeat((ucls % 16).astype(np.int64), per_core)
    # window assignment: class c windows dealt round-robin to cores
    sched_win = np.full((NCORE, NWp), -1, np.int64)  # global window id or -1
    base = np.concatenate([[0], np.cumsum(per_core)[:-1]])
    j_of = np.repeat(np.arange(len(ucls)), ncls)
    i_of = np.arange(len(order)) - np.repeat(starts, ncls)
    sched_win[i_of % NCORE, base[j_of] + i_of // NCORE] = order

    # groups of GRP windows
    ngrp = _ceil(NWp, GRP)
    groups = []
    sA = sB = 0
    boff = 0
    for g in range(ngrp):
        w0, w1 = g * GRP, min((g + 1) * GRP, NWp)
        gi = {"w0": w0, "w1": w1, "aoff": sA, "boff": sB, "btoff": boff,
              "tA": sched_tA[w0:w1], "tB": sched_tB[w0:w1]}
        gi["na"] = int(gi["tA"].sum())       # A tiles in group
        gi["nb"] = int(gi["tB"].sum())
        sA += gi["na"]
        sB += gi["nb"]
        boff += gi["na"] + gi["nb"]
        groups.append(gi)
    p.groups = groups
    p.ngrp = ngrp
    p.NWp = NWp
    p.R = NWp * WIN
    p.R128 = _ceil(p.R, 128) * 128
    p.ntA = sA                                # total A tiles per core
    p.ntB = sB
    p.nt = sA + sB
    p.slotsA, p.slotsB = sA * 128, sB * 128
    p.sched_tA, p.sched_tB, p.sched_win = sched_tA, sched_tB, sched_win
    # per-sched-window slot offsets (tile index) within each half stream
    p.atile0 = np.concatenate([[0], np.cumsum(sched_tA)[:-1]])
    p.btile0 = np.concatenate([[0], np.cumsum(sched_tB)[:-1]])
    p.g_aoff = np.array([g["aoff"] for g in groups])
    p.g_boff = np.array([g["boff"] for g in groups])
    p.g_btoff = np.array([g["btoff"] for g in groups])
    p.g_na = np.array([g["na"] for g in groups])
    return p


def build_core_inputs(p, k, rows, cols, vals):
    """idx streams (A,B) and B-matrix stream for core k. vals unscaled."""
    win = rows // WIN
    isA = cols < HALF
    # map global window -> sched pos for this core
    w2s = np.full(NWING, -1, np.int64)
    mine = p.sched_win[k] >= 0
    w2s[p.sched_win[k][mine]] = np.nonzero(mine)[0]
    m = w2s[win] >= 0
    r, c, v, a = rows[m], cols[m], vals[m], isA[m]
    s = w2s[r // WIN]                             # sched window position
    lr = r % WIN                                  # local row in window
    idxA = np.zeros(p.slotsA, np.int16)
    idxB = np.zeros(p.slotsB, np.int16)
    bmat = np.zeros((128, p.nt * WIN), np.float32)  # [slot%128, tile*32+lr]
    for half in (0, 1):
        hm = a if half == 0 else ~a
        ss, cc, vv, ll = s[hm], c[hm], v[hm], lr[hm]
        o = np.argsort(ss, kind="stable")
        ss, cc, vv, ll = ss[o], cc[o], vv[o], ll[o]
        first = np.searchsorted(ss, ss, side="left")
        rank = np.arange(len(ss)) - first
        tile0 = (p.atile0 if half == 0 else p.btile0)[ss]
        slot = tile0 * 128 + rank                 # slot within half stream
        if half == 0:
            idxA[slot] = cc.astype(np.int16)
        else:
            idxB[slot] = (cc - HALF).astype(np.int16)
        # B matrix: tile id in the COMBINED per-group stream order
        tile_h = tile0 + rank // 128              # tile within half stream
        gidx = ss // GRP
        if half == 0:
            btile = p.g_btoff[gidx] + (tile_h - p.g_aoff[gidx])
        else:
            btile = p.g_btoff[gidx] + p.g_na[gidx] + (tile_h - p.g_boff[gidx])
        bmat[slot % 128, btile * WIN + ll] = vv
    def wrap(arr):
        a2 = arr.reshape(-1, 16).T                 # [16, slots/16]
        return np.tile(a2, (8, 1)).astype(np.int16)
    return wrap(idxA), wrap(idxB), bmat


def _spmm_body(nc, tc, p, xsrc, idxA_sb, idxB_sb, bmat, xp_sb, dt, yp,
               ein=None, bm_res=None):
    """One SpMM rep; if ein=(t_sb, w_sb, on_sb, bv_sb, outt) also emits the
    einsum per group as soon as that group's xn columns are final.
    bm_res: SBUF-resident full B matrix (skips per-group B streaming)."""
    from concourse import mybir as mb
    from contextlib import ExitStack
    with ExitStack() as stk:
        gp = stk.enter_context(tc.tile_pool(name="g",
                                            bufs=2 if bm_res is not None else 3))
        bp = stk.enter_context(tc.tile_pool(name="bm", bufs=3))
        psp = stk.enter_context(tc.tile_pool(name="ps", bufs=4, space="PSUM"))
        if ein is not None:
            epsp = stk.enter_context(tc.tile_pool(name="eps", bufs=2,
                                                  space="PSUM"))
            osp = stk.enter_context(tc.tile_pool(name="eo", bufs=3))
            t_sb, w_sb, on_sb, bv_sb, outt = ein
        maxna = max(g["na"] for g in p.groups)
        maxnb = max(g["nb"] for g in p.groups)
        maxnt = max(g["na"] + g["nb"] for g in p.groups)
        xn = yp.tile([128, p.R128], dt.bfloat16, tag="xn")
        if p.R128 > p.R:
            nc.vector.memset(xn[:, p.R:], 0.0)
        ntile_e = p.R128 // 128
        etile_done = 0
        for gi, g in enumerate(p.groups):
            nw = g["w1"] - g["w0"]
            gA = gp.tile([128, maxna, 128], dt.bfloat16, tag="gA")
            nc.gpsimd.dma_gather(
                gA[:, :g["na"], :], xsrc[0:HALF, :],
                idxA_sb[:, g["aoff"] * 8:(g["aoff"] + g["na"]) * 8],
                g["na"] * 128, g["na"] * 128, 128, transpose=False,
                single_packet=False, queue_num=(2 * gi) % NQUEUES)
            if g["nb"]:
                gB = gp.tile([128, maxnb, 128], dt.bfloat16, tag="gB")
                nc.gpsimd.dma_gather(
                    gB[:, :g["nb"], :], xsrc[HALF:MP, :],
                    idxB_sb[:, g["boff"] * 8:(g["boff"] + g["nb"]) * 8],
                    g["nb"] * 128, g["nb"] * 128, 128, transpose=False,
                    single_packet=False, queue_num=(2 * gi + 1) % NQUEUES)
            nt_g = g["na"] + g["nb"]
            if bm_res is not None:
                bm = bm_res[:, g["btoff"] * WIN:]
            else:
                bm = bp.tile([128, maxnt * WIN], dt.bfloat16, tag="bm")
                nc.sync.dma_start(
                    bm[:, :nt_g * WIN],
                    bmat[:, g["btoff"] * WIN:(g["btoff"] + nt_g) * WIN])
            ps = psp.tile([128, GRP * WIN], dt.float32, tag="ps")
            for wl in range(nw):
                w = g["w0"] + wl
                ta, tb = int(p.sched_tA[w]), int(p.sched_tB[w])
                a0 = int(p.atile0[w] - g["aoff"])
                b0 = int(p.btile0[w] - g["boff"])
                nmm = ta + tb
                done = 0
                pslice = ps[:, wl * WIN:(wl + 1) * WIN]
                for j in range(ta):
                    nc.tensor.matmul(
                        pslice, gA[:, a0 + j, :],
                        bm[:, (a0 + j) * WIN:(a0 + j + 1) * WIN],
                        start=(done == 0), stop=(done == nmm - 1))
                    done += 1
                for j in range(tb):
                    bt = g["na"] + b0 + j
                    nc.tensor.matmul(
                        pslice, gB[:, b0 + j, :],
                        bm[:, bt * WIN:(bt + 1) * WIN],
                        start=(done == 0), stop=(done == nmm - 1))
                    done += 1
            c0 = g["w0"] * WIN
            c1 = g["w1"] * WIN
            nc.vector.tensor_tensor(xn[:, c0:c1], ps[:, :(c1 - c0)],
                                    xp_sb[:, c0:c1], mb.AluOpType.subtract)
            if ein is not None:
                lim = min((c1 if gi < p.ngrp - 1 else p.R128) // 128, ntile_e)
                t_all = t_sb + [xn]
                for t in range(etile_done, lim):
                    eps = epsp.tile([128, 256], dt.float32, tag="eps")
                    for kk in range(KCH):
                        nc.tensor.matmul(
                            eps[:], t_all[kk][:, t * 128:(t + 1) * 128],
                            w_sb[kk][:], start=(kk == 0), stop=False)
                    nc.tensor.matmul(eps[:], on_sb[:], bv_sb[:],
                                     start=False, stop=True)
                    o = osp.tile([128, 256], dt.bfloat16, tag="eo")
                    nc.vector.tensor_copy(o[:], eps[:])
                    nc.sync.dma_start(outt[t * 128:(t + 1) * 128, :], o[:])
                etile_done = lim
    return xn


def build_spmm_nc(p, reps=1, fuse_einsum=False, timing_mode=False):
    """timing_mode: big read-only tensors become DRAM scratch (garbage data,
    identical instruction stream/addressing) so per-call input copies vanish
    and chained-dispatch rep-deltas measure pure device execution."""
    nc = bacc.Bacc("TRN2", num_swdge_queues=NQUEUES)
    dt = mybir.dt

    def big_in(name, shape, dtype):
        if timing_mode:
            return nc.dram_tensor(name, shape, dtype, kind="Internal")
        return nc.dram_tensor(name, shape, dtype, kind="ExternalInput")

    xsrc = big_in("xsrc", [MP, 128], dt.bfloat16)
    if not timing_mode:
        idxA = nc.dram_tensor("idxA", [128, p.slotsA // 16], dt.int16,
                              kind="ExternalInput")
        idxB = nc.dram_tensor("idxB", [128, p.slotsB // 16], dt.int16,
                              kind="ExternalInput")
    bmat = big_in("bmat", [128, p.nt * WIN], dt.bfloat16)
    xprev = nc.dram_tensor("xprev", [128, p.R128], dt.bfloat16,
                           kind="ExternalInput")
    if fuse_einsum:
        ts = [big_in(f"t{i}", [128, p.R128], dt.bfloat16)
              for i in range(KCH - 1)]
        wm = big_in("wm", [KCH * 128, 256], dt.bfloat16)
        bvec = big_in("bvec", [1, 256], dt.bfloat16)
        outt = nc.dram_tensor("outt", [p.R128, 256], dt.bfloat16,
                              kind="ExternalOutput")
    else:
        xnext = nc.dram_tensor("xnext", [128, p.R128], dt.bfloat16,
                               kind="ExternalOutput")
    with TileContext(nc) as tc:
        nc.gpsimd.load_library(mlp)
        with tc.tile_pool(name="io", bufs=1) as iop:
            idxA_sb = iop.tile([128, p.slotsA // 16], dt.int16)
            idxB_sb = iop.tile([128, p.slotsB // 16], dt.int16)
            if timing_mode:
                # synth pseudo-random valid indices: (i*37) & 16383
                for sb, nfree in ((idxA_sb, p.slotsA // 16),
                                  (idxB_sb, p.slotsB // 16)):
                    t32 = iop.tile([128, nfree], dt.int32, tag="t32")
                    nc.gpsimd.iota(t32[:], [[1, nfree]], base=0,
                                   channel_multiplier=nfree)
                    nc.vector.tensor_scalar(t32[:], t32[:], 37, None,
                                            op0=mybir.AluOpType.mult)
                    nc.vector.tensor_scalar(t32[:], t32[:], 16383, None,
                                            op0=mybir.AluOpType.bitwise_and)
                    nc.vector.tensor_copy(sb[:], t32[:])
            else:
                nc.sync.dma_start(idxA_sb[:], idxA[:])
                nc.sync.dma_start(idxB_sb[:], idxB[:])
            xp_sb = iop.tile([128, p.R128], dt.bfloat16)
            nc.sync.dma_start(xp_sb[:], xprev[:])
            if fuse_einsum:
                t_sb = []
                for i in range(KCH - 1):
                    tt = iop.tile([128, p.R128], dt.bfloat16, tag=f"t{i}")
                    nc.sync.dma_start(tt[:], ts[i][:])
                    t_sb.append(tt)
                w_sb = []
                for i in range(KCH):
                    wt = iop.tile([128, 256], dt.bfloat16, tag=f"w{i}")
                    nc.sync.dma_start(wt[:], wm[i * 128:(i + 1) * 128, :])
                    w_sb.append(wt)
                on_sb = iop.tile([1, 128], dt.bfloat16)
                nc.vector.memset(on_sb[:], 1.0)
                bv_sb = iop.tile([1, 256], dt.bfloat16)
                nc.sync.dma_start(bv_sb[:], bvec[:])
            from contextlib import ExitStack
            _stk = ExitStack()
            yp = _stk.enter_context(tc.tile_pool(name="y", bufs=2))
            ein = (t_sb, w_sb, on_sb, bv_sb, outt) if fuse_einsum else None
            bm_res = None
            if not fuse_einsum:
                # B fits in SBUF when the einsum tensors aren't resident:
                # keep it on-chip so gather SWDGE queues don't share SDMA
                # packet slots with a per-group HWDGE B stream.
                bm_res_t = iop.tile([128, p.nt * WIN], dt.bfloat16, tag="bmr")
                nc.sync.dma_start(bm_res_t[:], bmat[:])
                bm_res = bm_res_t
            for rr in range(reps):
                xn = _spmm_body(nc, tc, p, xsrc, idxA_sb, idxB_sb, bmat,
                                xp_sb, dt, yp, ein=ein, bm_res=bm_res)
                if not fuse_einsum:
                    nc.sync.dma_start(xnext[:, :], xn[:])
            _stk.close()
    nc.compile()
    return nc


_CACHE = {}


def _run(nc, in_maps):
    return run_bass_kernel_spmd(nc, in_maps, core_ids=list(range(NCORE)))


def _get_ncs(p):
    key = (p.nt, p.R128)
    if key not in _CACHE:
        _CACHE[key] = (build_spmm_nc(p), build_spmm_nc(p, fuse_einsum=True))
    return _CACHE[key]


def make_wmat(w, b):
    wmat = np.zeros((KCH * 128, 256), np.float32)
    for k in range(KCH):
        for pp in range(128):
            f, n = pp // 8, pp % 8
            wmat[k * 128 + pp, n * 32:(n + 1) * 32] = w[f, k, :]
    bv = np.tile(b.reshape(1, FOUT), (1, 8)).astype(np.float32)
    return wmat.astype(BF16), bv.astype(BF16)


def kernel(x, l_vals, w, b, l_row, l_col, _timing=None):
    x = np.asarray(x, np.float32)
    l_vals = np.asarray(l_vals, np.float32)
    w = np.asarray(w, np.float32)
    b = np.asarray(b, np.float32)
    rows = np.asarray(l_row).astype(np.int64)
    cols = np.asarray(l_col).astype(np.int64)

    p = build_plan(rows, cols)
    nc_spmm, nc_l3 = _get_ncs(p)

    X0 = np.zeros((MP, 128), np.float32)
    X0[:M] = x.transpose(1, 2, 0).reshape(M, FIN * N)

    core_in = [build_core_inputs(p, k, rows, cols, l_vals) for k in range(NCORE)]

    # sched row -> global row map per core (for slicing/assembly)
    rowmap = []     # (sched positions with real windows, global row indices)
    for k in range(NCORE):
        mine = np.nonzero(p.sched_win[k] >= 0)[0]
        gw = p.sched_win[k][mine]
        smat = (mine[:, None] * WIN + np.arange(WIN)[None, :]).ravel()
        gmat = (gw[:, None] * WIN + np.arange(WIN)[None, :]).ravel()
        keep = gmat < M
        rowmap.append((smat[keep], gmat[keep]))

    def dev_slices(Xfull):
        out = []
        for k in range(NCORE):
            s = np.zeros((128, p.R128), BF16)
            sp, gp_ = rowmap[k]
            s[:, sp] = Xfull[gp_].T.astype(BF16)
            out.append(s)
        return out

    def assemble(slices):
        Xf = np.zeros((MP, 128), np.float32)
        for k in range(NCORE):
            sp, gp_ = rowmap[k]
            Xf[gp_] = slices[k][:, sp].T.astype(np.float32)
        return Xf

    import time
    times = []
    Xt_slices = [dev_slices(X0)]
    Xcur = X0
    zeros_sl = [np.zeros((128, p.R128), BF16)] * NCORE
    wmat, bv = make_wmat(w, b)

    for it in range(KCH - 1):
        alpha = 1.0 if it == 0 else 2.0
        xprev_sl = zeros_sl if it == 0 else Xt_slices[it - 1]
        in_maps = [{
            "xsrc": Xcur.astype(BF16),
            "idxA": core_in[k][0],
            "idxB": core_in[k][1],
            "bmat": (alpha * core_in[k][2]).astype(BF16),
            "xprev": xprev_sl[k],
        } for k in range(NCORE)]
        if it < KCH - 2:
            t0 = time.time()
            res = _run(nc_spmm, in_maps)
            times.append(time.time() - t0)
            new_sl = [res.results[k]["xnext"] for k in range(NCORE)]
            Xt_slices.append(new_sl)
            Xcur = assemble(new_sl)
        else:
            for k in range(NCORE):
                in_maps[k].update({
                    **{f"t{i}": Xt_slices[i][k] for i in range(KCH - 1)},
                    "wm": wmat, "bvec": bv,
                })
            t0 = time.time()
            res = _run(nc_l3, in_maps)
            times.append(time.time() - t0)

    out = np.zeros((N, M, FOUT), np.float32)
    for k in range(NCORE):
        o = np.asarray(res.results[k]["outt"], np.float32)   # [R128, 256]
        sp, gp_ = rowmap[k]
        o3 = o[sp].reshape(-1, N, FOUT)
        out[:, gp_, :] = o3.transpose(1, 0, 2)
    if _timing is not None:
        _timing.extend(times)
    return out


# revision 5
# speedup vs baseline: 1.2050x; 1.2050x over previous
"""Trainium2 Bass kernel for nn_ChebyshevLayer (gnn_message_passing) — v3 (PE SpMM).

Sharding/structure (8 NeuronCores, SPMD, 3 launches):
- X0 = transpose(x,(1,2,0)).reshape(M, Fin*N) -> [50000, 128] bf16, padded to
  50176 rows; kept in HBM in natural row-major layout (256B rows).
- Rows grouped in 32-row WINDOWS (1568 global). Window w needs its nnz's
  X[col] rows; cols split in halves (int16 gather range). Per (window, half)
  the slots are padded to a multiple of 128 (tiles). Windows are dealt to the
  8 cores BY CLASS (tA, tB) so every core runs an identical schedule (one
  NEFF); per-class remainders are padded with dummy windows.
- SpMM: per group of 16 windows (one PSUM bank [128,512] f32): two
  dma_gathers (half A / half B, natural layout: slot->partition, 4 SWDGE
  queues round-robin), then per 128-slot tile one PE matmul
  psum[feat, 32] += G_tile^T[feat, slots] @ B_tile[slots, 32], where
  B[s, r] = alpha*val[s] if slot s targets local row r (bf16, streamed).
  DVE then computes xn[:, grp] = psum - xprev[:, grp] (Chebyshev recurrence;
  alpha=2 folded into vals for iterations 2,3; xprev=0 on iteration 1).
- Launch 3 additionally fuses the einsum: psum[m,256] = sum_k T_k^T @ What_k
  (block-diag What: [(f,n), (n,o)]) + ones^T@bias, out bf16 [R,256].
- Host assembles the full X between launches (pure data movement).
"""

import numpy as np
import ml_dtypes

import concourse.bacc as bacc
import concourse.mybir as mybir
from concourse.tile import TileContext
from concourse.bass_utils import run_bass_kernel_spmd
from concourse.library_config import mlp

N, M, FIN, FOUT, KCH = 8, 50000, 16, 32, 4
WIN = 44                   # rows per window
NWING = -(-M // WIN)       # global windows
MP = NWING * WIN           # padded rows
HALF = MP // 2             # col half for int16 gather indices
NCORE = 8
NQUEUES = 4
GRP = 11                   # windows per psum bank group (GRP*WIN*4B <= 2KB)
BF16 = ml_dtypes.bfloat16


def _ceil(a, b):
    return -(-a // b)


class Plan:
    pass


def build_plan(rows, cols):
    """Window classes, per-core schedule (shared shape), slot/B layout."""
    p = Plan()
    win = rows // WIN
    isA = cols < HALF
    cntA = np.bincount(win[isA], minlength=NWING).astype(np.int64)
    cntB = np.bincount(win[~isA], minlength=NWING).astype(np.int64)
    tA = np.maximum(1, _ceil(cntA, 128))     # >=1 so every window is written
    tB = _ceil(cntB, 128)
    cls = tA * 16 + tB
    order = np.argsort(cls, kind="stable")
    ucls, starts, ncls = np.unique(cls[order], return_index=True,
                                   return_counts=True)
    per_core = _ceil(ncls, NCORE)            # windows of this class per core
    NWp = int(per_core.sum())                # scheduled windows per core
    # schedule (same for all cores): sched position s -> (tA, tB)
    sched_tA = np.repeat((ucls // 16).astype(np.int64), per_core)
    sched_tB = np.repeat((ucls % 16).astype(np.int64), per_core)
    # window assignment: class c windows dealt round-robin to cores
    sched_win = np.full((NCORE, NWp), -1, np.int64)  # global window id or -1
    base = np.concatenate([[0], np.cumsum(per_core)[:-1]])
    j_of = np.repeat(np.arange(len(ucls)), ncls)
    i_of = np.arange(len(order)) - np.repeat(starts, ncls)
    sched_win[i_of % NCORE, base[j_of] + i_of // NCORE] = order

    # groups of GRP windows
    ngrp = _ceil(NWp, GRP)
    groups = []
    sA = sB = 0
    boff = 0
    for g in range(ngrp):
        w0, w1 = g * GRP, min((g + 1) * GRP, NWp)
        gi = {"w0": w0, "w1": w1, "aoff": sA, "boff": sB, "btoff": boff,
              "tA": sched_tA[w0:w1], "tB": sched_tB[w0:w1]}
        gi["na"] = int(gi["tA"].sum())       # A tiles in group
        gi["nb"] = int(gi["tB"].sum())
        sA += gi["na"]
        sB += gi["nb"]
        boff += gi["na"] + gi["nb"]
        groups.append(gi)
    p.groups = groups
    p.ngrp = ngrp
    p.NWp = NWp
    p.R = NWp * WIN
    p.R128 = _ceil(p.R, 128) * 128
    p.ntA = sA                                # total A tiles per core
    p.ntB = sB
    p.nt = sA + sB
    p.slotsA, p.slotsB = sA * 128, sB * 128
    p.sched_tA, p.sched_tB, p.sched_win = sched_tA, sched_tB, sched_win
    # per-sched-window slot offsets (tile index) within each half stream
    p.atile0 = np.concatenate([[0], np.cumsum(sched_tA)[:-1]])
    p.btile0 = np.concatenate([[0], np.cumsum(sched_tB)[:-1]])
    p.g_aoff = np.array([g["aoff"] for g in groups])
    p.g_boff = np.array([g["boff"] for g in groups])
    p.g_btoff = np.array([g["btoff"] for g in groups])
    p.g_na = np.array([g["na"] for g in groups])
    return p


def build_core_inputs(p, k, rows, cols, vals):
    """idx streams (A,B) and B-matrix stream for core k. vals unscaled."""
    win = rows // WIN
    isA = cols < HALF
    # map global window -> sched pos for this core
    w2s = np.full(NWING, -1, np.int64)
    mine = p.sched_win[k] >= 0
    w2s[p.sched_win[k][mine]] = np.nonzero(mine)[0]
    m = w2s[win] >= 0
    r, c, v, a = rows[m], cols[m], vals[m], isA[m]
    s = w2s[r // WIN]                             # sched window position
    lr = r % WIN                                  # local row in window
    idxA = np.zeros(p.slotsA, np.int16)
    idxB = np.zeros(p.slotsB, np.int16)
    bmat = np.zeros((128, p.nt * WIN), np.float32)  # [slot%128, tile*32+lr]
    for half in (0, 1):
        hm = a if half == 0 else ~a
        ss, cc, vv, ll = s[hm], c[hm], v[hm], lr[hm]
        o = np.argsort(ss, kind="stable")
        ss, cc, vv, ll = ss[o], cc[o], vv[o], ll[o]
        first = np.searchsorted(ss, ss, side="left")
        rank = np.arange(len(ss)) - first
        tile0 = (p.atile0 if half == 0 else p.btile0)[ss]
        slot = tile0 * 128 + rank                 # slot within half stream
        if half == 0:
            idxA[slot] = cc.astype(np.int16)
        else:
            idxB[slot] = (cc - HALF).astype(np.int16)
        # B matrix: tile id in the COMBINED per-group stream order
        tile_h = tile0 + rank // 128              # tile within half stream
        gidx = ss // GRP
        if half == 0:
            btile = p.g_btoff[gidx] + (tile_h - p.g_aoff[gidx])
        else:
            btile = p.g_btoff[gidx] + p.g_na[gidx] + (tile_h - p.g_boff[gidx])
        bmat[slot % 128, btile * WIN + ll] = vv
    def wrap(arr):
        a2 = arr.reshape(-1, 16).T                 # [16, slots/16]
        return np.tile(a2, (8, 1)).astype(np.int16)
    return wrap(idxA), wrap(idxB), bmat


def _spmm_body(nc, tc, p, xsrc, idxA_sb, idxB_sb, bmat, xp_sb, dt, yp,
               ein=None, bm_res=None):
    """One SpMM rep; if ein=(t_sb, w_sb, on_sb, bv_sb, outt) also emits the
    einsum per group as soon as that group's xn columns are final.
    bm_res: SBUF-resident full B matrix (skips per-group B streaming)."""
    from concourse import mybir as mb
    from contextlib import ExitStack
    with ExitStack() as stk:
        gp = stk.enter_context(tc.tile_pool(name="g",
                                            bufs=2 if bm_res is not None else 3))
        bp = stk.enter_context(tc.tile_pool(name="bm", bufs=3))
        psp = stk.enter_context(tc.tile_pool(name="ps", bufs=4, space="PSUM"))
        if ein is not None:
            epsp = stk.enter_context(tc.tile_pool(name="eps", bufs=2,
                                                  space="PSUM"))
            osp = stk.enter_context(tc.tile_pool(name="eo", bufs=3))
            t_sb, w_sb, on_sb, bv_sb, outt = ein
        maxna = max(g["na"] for g in p.groups)
        maxnb = max(g["nb"] for g in p.groups)
        maxnt = max(g["na"] + g["nb"] for g in p.groups)
        xn = yp.tile([128, p.R128], dt.bfloat16, tag="xn")
        if p.R128 > p.R:
            nc.vector.memset(xn[:, p.R:], 0.0)
        ntile_e = p.R128 // 128
        etile_done = 0
        for gi, g in enumerate(p.groups):
            nw = g["w1"] - g["w0"]
            gA = gp.tile([128, maxna, 128], dt.bfloat16, tag="gA")
            nc.gpsimd.dma_gather(
                gA[:, :g["na"], :], xsrc[0:HALF, :],
                idxA_sb[:, g["aoff"] * 8:(g["aoff"] + g["na"]) * 8],
                g["na"] * 128, g["na"] * 128, 128, transpose=False,
                single_packet=False, queue_num=(2 * gi) % NQUEUES)
            if g["nb"]:
                gB = gp.tile([128, maxnb, 128], dt.bfloat16, tag="gB")
                nc.gpsimd.dma_gather(
                    gB[:, :g["nb"], :], xsrc[HALF:MP, :],
                    idxB_sb[:, g["boff"] * 8:(g["boff"] + g["nb"]) * 8],
                    g["nb"] * 128, g["nb"] * 128, 128, transpose=False,
                    single_packet=False, queue_num=(2 * gi + 1) % NQUEUES)
            nt_g = g["na"] + g["nb"]
            if bm_res is not None:
                bm = bm_res[:, g["btoff"] * WIN:]
            else:
                bm = bp.tile([128, maxnt * WIN], dt.bfloat16, tag="bm")
                nc.sync.dma_start(
                    bm[:, :nt_g * WIN],
                    bmat[:, g["btoff"] * WIN:(g["btoff"] + nt_g) * WIN])
            ps = psp.tile([128, GRP * WIN], dt.float32, tag="ps")
            for wl in range(nw):
                w = g["w0"] + wl
                ta, tb = int(p.sched_tA[w]), int(p.sched_tB[w])
                a0 = int(p.atile0[w] - g["aoff"])
                b0 = int(p.btile0[w] - g["boff"])
                nmm = ta + tb
                done = 0
                pslice = ps[:, wl * WIN:(wl + 1) * WIN]
                for j in range(ta):
                    nc.tensor.matmul(
                        pslice, gA[:, a0 + j, :],
                        bm[:, (a0 + j) * WIN:(a0 + j + 1) * WIN],
                        start=(done == 0), stop=(done == nmm - 1))
                    done += 1
                for j in range(tb):
                    bt = g["na"] + b0 + j
                    nc.tensor.matmul(
                        pslice, gB[:, b0 + j, :],
                        bm[:, bt * WIN:(bt + 1) * WIN],
                        start=(done == 0), stop=(done == nmm - 1))
                    done += 1
            c0 = g["w0"] * WIN
            c1 = g["w1"] * WIN
            nc.vector.tensor_tensor(xn[:, c0:c1], ps[:, :(c1 - c0)],
                                    xp_sb[:, c0:c1], mb.AluOpType.subtract)
            if ein is not None:
                lim = min((c1 if gi < p.ngrp - 1 else p.R128) // 128, ntile_e)
                t_all = t_sb + [xn]
                for t in range(etile_done, lim):
                    eps = epsp.tile([128, 256], dt.float32, tag="eps")
                    for kk in range(KCH):
                        nc.tensor.matmul(
                            eps[:], t_all[kk][:, t * 128:(t + 1) * 128],
                            w_sb[kk][:], start=(kk == 0), stop=False)
                    nc.tensor.matmul(eps[:], on_sb[:], bv_sb[:],
                                     start=False, stop=True)
                    o = osp.tile([128, 256], dt.bfloat16, tag="eo")
                    nc.vector.tensor_copy(o[:], eps[:])
                    nc.sync.dma_start(outt[t * 128:(t + 1) * 128, :], o[:])
                etile_done = lim
    return xn


def build_spmm_nc(p, reps=1, fuse_einsum=False, timing_mode=False):
    """timing_mode: big read-only tensors become DRAM scratch (garbage data,
    identical instruction stream/addressing) so per-call input copies vanish
    and chained-dispatch rep-deltas measure pure device execution."""
    nc = bacc.Bacc("TRN2", num_swdge_queues=NQUEUES)
    dt = mybir.dt

    def big_in(name, shape, dtype):
        if timing_mode:
            return nc.dram_tensor(name, shape, dtype, kind="Internal")
        return nc.dram_tensor(name, shape, dtype, kind="ExternalInput")

    xsrc = big_in("xsrc", [MP, 128], dt.bfloat16)
    if not timing_mode:
        idxA = nc.dram_tensor("idxA", [128, p.slotsA // 16], dt.int16,
                              kind="ExternalInput")
        idxB = nc.dram_tensor("idxB", [128, p.slotsB // 16], dt.int16,
                              kind="ExternalInput")
    bmat = big_in("bmat", [128, p.nt * WIN], dt.bfloat16)
    xprev = nc.dram_tensor("xprev", [128, p.R128], dt.bfloat16,
                           kind="ExternalInput")
    if fuse_einsum:
        ts = [big_in(f"t{i}", [128, p.R128], dt.bfloat16)
              for i in range(KCH - 1)]
        wm = big_in("wm", [KCH * 128, 256], dt.bfloat16)
        bvec = big_in("bvec", [1, 256], dt.bfloat16)
        outt = nc.dram_tensor("outt", [p.R128, 256], dt.bfloat16,
                              kind="ExternalOutput")
    else:
        xnext = nc.dram_tensor("xnext", [128, p.R128], dt.bfloat16,
                               kind="ExternalOutput")
    with TileContext(nc) as tc:
        nc.gpsimd.load_library(mlp)
        with tc.tile_pool(name="io", bufs=1) as iop:
            idxA_sb = iop.tile([128, p.slotsA // 16], dt.int16)
            idxB_sb = iop.tile([128, p.slotsB // 16], dt.int16)
            if timing_mode:
                # synth pseudo-random valid indices: (i*37) & 16383
                for sb, nfree in ((idxA_sb, p.slotsA // 16),
                                  (idxB_sb, p.slotsB // 16)):
                    t32 = iop.tile([128, nfree], dt.int32, tag="t32")
                    nc.gpsimd.iota(t32[:], [[1, nfree]], base=0,
                                   channel_multiplier=nfree)
                    nc.vector.tensor_scalar(t32[:], t32[:], 37, None,
                                            op0=mybir.AluOpType.mult)
                    nc.vector.tensor_scalar(t32[:], t32[:], 16383, None,
                                            op0=mybir.AluOpType.bitwise_and)
                    nc.vector.tensor_copy(sb[:], t32[:])
            else:
                nc.sync.dma_start(idxA_sb[:], idxA[:])
                nc.sync.dma_start(idxB_sb[:], idxB[:])
            xp_sb = iop.tile([128, p.R128], dt.bfloat16)
            nc.sync.dma_start(xp_sb[:], xprev[:])
            if fuse_einsum:
                t_sb = []
                for i in range(KCH - 1):
                    tt = iop.tile([128, p.R128], dt.bfloat16, tag=f"t{i}")
                    nc.sync.dma_start(tt[:], ts[i][:])
                    t_sb.append(tt)
                w_sb = []
                for i in range(KCH):
                    wt = iop.tile([128, 256], dt.bfloat16, tag=f"w{i}")
                    nc.sync.dma_start(wt[:], wm[i * 128:(i + 1) * 128, :])
                    w_sb.append(wt)
                on_sb = iop.tile([1, 128], dt.bfloat16)
                nc.vector.memset(on_sb[:], 1.0)
                bv_sb = iop.tile([1, 256], dt.bfloat16)
                nc.sync.dma_start(bv_sb[:], bvec[:])
            from contextlib import ExitStack
            _stk = ExitStack()
            yp = _stk.enter_context(tc.tile_pool(name="y", bufs=2))
            ein = (t_sb, w_sb, on_sb, bv_sb, outt) if fuse_einsum else None
            # B-resident variant measured slower on HW (shallower gather
            # buffering outweighs removing the B stream) — keep streaming.
            bm_res = None
            for rr in range(reps):
                xn = _spmm_body(nc, tc, p, xsrc, idxA_sb, idxB_sb, bmat,
                                xp_sb, dt, yp, ein=ein, bm_res=bm_res)
                if not fuse_einsum:
                    nc.sync.dma_start(xnext[:, :], xn[:])
            _stk.close()
    nc.compile()
    return nc


_CACHE = {}


def _run(nc, in_maps):
    return run_bass_kernel_spmd(nc, in_maps, core_ids=list(range(NCORE)))


def _get_ncs(p):
    key = (p.nt, p.R128)
    if key not in _CACHE:
        _CACHE[key] = (build_spmm_nc(p), build_spmm_nc(p, fuse_einsum=True))
    return _CACHE[key]


def make_wmat(w, b):
    wmat = np.zeros((KCH * 128, 256), np.float32)
    for k in range(KCH):
        for pp in range(128):
            f, n = pp // 8, pp % 8
            wmat[k * 128 + pp, n * 32:(n + 1) * 32] = w[f, k, :]
    bv = np.tile(b.reshape(1, FOUT), (1, 8)).astype(np.float32)
    return wmat.astype(BF16), bv.astype(BF16)


def kernel(x, l_vals, w, b, l_row, l_col, _timing=None):
    x = np.asarray(x, np.float32)
    l_vals = np.asarray(l_vals, np.float32)
    w = np.asarray(w, np.float32)
    b = np.asarray(b, np.float32)
    rows = np.asarray(l_row).astype(np.int64)
    cols = np.asarray(l_col).astype(np.int64)

    p = build_plan(rows, cols)
    nc_spmm, nc_l3 = _get_ncs(p)

    X0 = np.zeros((MP, 128), np.float32)
    X0[:M] = x.transpose(1, 2, 0).reshape(M, FIN * N)

    core_in = [build_core_inputs(p, k, rows, cols, l_vals) for k in range(NCORE)]

    # sched row -> global row map per core (for slicing/assembly)
    rowmap = []     # (sched positions with real windows, global row indices)
    for k in range(NCORE):
        mine = np.nonzero(p.sched_win[k] >= 0)[0]
        gw = p.sched_win[k][mine]
        smat = (mine[:, None] * WIN + np.arange(WIN)[None, :]).ravel()
        gmat = (gw[:, None] * WIN + np.arange(WIN)[None, :]).ravel()
        keep = gmat < M
        rowmap.append((smat[keep], gmat[keep]))

    def dev_slices(Xfull):
        out = []
        for k in range(NCORE):
            s = np.zeros((128, p.R128), BF16)
            sp, gp_ = rowmap[k]
            s[:, sp] = Xfull[gp_].T.astype(BF16)
            out.append(s)
        return out

    def assemble(slices):
        Xf = np.zeros((MP, 128), np.float32)
        for k in range(NCORE):
            sp, gp_ = rowmap[k]
            Xf[gp_] = slices[k][:, sp].T.astype(np.float32)
        return Xf

    import time
    times = []
    Xt_slices = [dev_slices(X0)]
    Xcur = X0
    zeros_sl = [np.zeros((128, p.R128), BF16)] * NCORE
    wmat, bv = make_wmat(w, b)

    for it in range(KCH - 1):
        alpha = 1.0 if it == 0 else 2.0
        xprev_sl = zeros_sl if it == 0 else Xt_slices[it - 1]
        in_maps = [{
            "xsrc": Xcur.astype(BF16),
            "idxA": core_in[k][0],
            "idxB": core_in[k][1],
            "bmat": (alpha * core_in[k][2]).astype(BF16),
            "xprev": xprev_sl[k],
        } for k in range(NCORE)]
        if it < KCH - 2:
            t0 = time.time()
            res = _run(nc_spmm, in_maps)
            times.append(time.time() - t0)
            new_sl = [res.results[k]["xnext"] for k in range(NCORE)]
            Xt_slices.append(new_sl)
            Xcur = assemble(new_sl)
        else:
            for k in range(NCORE):
                in_maps[k].update({
                    **{f"t{i}": Xt_slices[i][k] for i in range(KCH - 1)},
                    "wm": wmat, "bvec": bv,
                })
            t0 = time.time()
            res = _run(nc_l3, in_maps)
            times.append(time.time() - t0)

    out = np.zeros((N, M, FOUT), np.float32)
    for k in range(NCORE):
        o = np.asarray(res.results[k]["outt"], np.float32)   # [R128, 256]
        sp, gp_ = rowmap[k]
        o3 = o[sp].reshape(-1, N, FOUT)
        out[:, gp_, :] = o3.transpose(1, 0, 2)
    if _timing is not None:
        _timing.extend(times)
    return out
